# revision 30
# baseline (speedup 1.0000x reference)
"""AdaptiveCenterLoss on 8 TRN2 NeuronCores.

loss = mean_i ||features[i] - centers[labels[i]]||^2
     with B=131072, D=256, C=1000.

Strategy (data-parallel, memory-bound):
  - host-side, sort rows by label and pack them into 8-row blocks, each
    block sharing one label; partial blocks are padded with rows equal to
    that class's center (contributing exactly 0 to the sum)
  - shard the padded blocks across 8 cores x 128 partitions as J 8-row
    "block-slots" per partition; per block-slot, ONE [128,1]-index
    indirect DMA gathers the 128 needed center rows (per-descriptor DGE
    cost makes per-row gathers ~16x more expensive on this HW)
  - block-slots are processed in pairs (one feature DMA + one DVE
    subtract with the two centers broadcast via a stride-0 4D AP + one
    ACT square+row-sum accumulate per pair); a trailing odd slot runs
    as a half tile, which also drains the pipeline faster
  - each core outputs per-tile partial sums; host sums and divides by B
"""

import numpy as np

import concourse.bacc as bacc
import concourse.bass as bass
import concourse.mybir as mybir
import concourse.tile as tile
from concourse.bass_utils import run_bass_kernel_spmd

B, D, C = 131072, 256, 1000
N_CORES = 8
P = 128   # SBUF partitions
S = 16    # rows per block (one label per block)

_nc_cache = {}


def _build(J):
    """Per-core graph for J block-slots per partition (J*8 rows each)."""
    if J in _nc_cache:
        return _nc_cache[J]
    splits = [1] * J
    acc_cols = sum(splits)

    nc = bacc.Bacc()
    feats = nc.declare_dram_parameter(
        "features", [J * P * S, D], mybir.dt.float32, isOutput=False
    )
    labels = nc.declare_dram_parameter("labels", [P, J], mybir.dt.int32, isOutput=False)
    centers = nc.declare_dram_parameter(
        "centers", [C, D], mybir.dt.float32, isOutput=False
    )
    out = nc.declare_dram_parameter(
        "out", [P, acc_cols], mybir.dt.float32, isOutput=True
    )

    # block-slot j, partition p, slot s <- feature row (j*128 + p)*8 + s
    fview = feats[:].rearrange("(j p s) d -> p j s d", p=P, s=S)

    with tile.TileContext(nc) as tc:
        with (
            tc.tile_pool(name="lab", bufs=1) as lab_pool,
            tc.tile_pool(name="f", bufs=6) as f_pool,
            tc.tile_pool(name="c", bufs=6) as c_pool,
            tc.tile_pool(name="acc", bufs=1) as acc_pool,
        ):
            lab = lab_pool.tile([P, J], mybir.dt.int32)
            nc.sync.dma_start(out=lab[:], in_=labels[:])
            acc = acc_pool.tile([P, acc_cols], mybir.dt.float32)
            col = 0
            for t in range(J):
                H = splits[t]
                SH = S // H
                f_t = f_pool.tile([P, S * D], mybir.dt.float32, tag="f")
                for h in range(H):
                    nc.sync.dma_start(
                        out=f_t[:, h * SH * D : (h + 1) * SH * D].rearrange(
                            "p (s d) -> p s d", s=SH
                        ),
                        in_=fview[:, t, h * SH : (h + 1) * SH, :],
                    )
                c_s = c_pool.tile([P, D], mybir.dt.float32, tag="c")
                nc.gpsimd.indirect_dma_start(
                    out=c_s[:],
                    out_offset=None,
                    in_=centers[:],
                    in_offset=bass.IndirectOffsetOnAxis(ap=lab[:, t : t + 1], axis=0),
                )
                c_b = (
                    c_s[:]
                    .rearrange("p (s d) -> p s d", s=1)
                    .to_broadcast([P, SH, D])
                )
                for h in range(H):
                    fh = f_t[:, h * SH * D : (h + 1) * SH * D]
                    nc.vector.tensor_tensor(
                        out=fh.rearrange("p (s d) -> p s d", s=SH),
                        in0=fh.rearrange("p (s d) -> p s d", s=SH),
                        in1=c_b,
                        op=mybir.AluOpType.subtract,
                    )
                    nc.scalar.activation(
                        out=fh,
                        in_=fh,
                        func=mybir.ActivationFunctionType.Square,
                        accum_out=acc[:, col : col + 1],
                    )
                    col += 1
            nc.sync.dma_start(out=out[:], in_=acc[:])
    nc.finalize()
    _nc_cache[J] = nc
    return nc


def _prepare(features, centers, labels):
    """Sort rows by label into padded S-row blocks; returns per-core maps + J."""
    features = np.ascontiguousarray(np.asarray(features), dtype=np.float32)
    centers = np.ascontiguousarray(np.asarray(centers), dtype=np.float32)
    labels = np.asarray(labels).astype(np.int32)

    counts = np.bincount(labels, minlength=C)          # [C]
    nblocks = -(-counts // S)                          # ceil(n_c / S) per class
    nb = int(nblocks.sum())
    group = N_CORES * P                                # blocks per slot across chip
    nb_pad = -(-nb // group) * group
    J = nb_pad // group                                # block-slots per partition

    # block labels, in sorted-class order; pad blocks use class 0
    block_labels = np.zeros(nb_pad, dtype=np.int32)
    block_labels[:nb] = np.repeat(np.arange(C, dtype=np.int32), nblocks)

    # every padded slot starts as its block's center row -> contributes 0
    fpad = centers[block_labels].repeat(S, axis=0).reshape(nb_pad * S, D)

    # scatter the real rows into their slots
    order = np.argsort(labels)
    labels_sorted = labels[order]
    class_row_start = np.concatenate(([0], np.cumsum(counts)[:-1]))
    class_slot_start = S * np.concatenate(([0], np.cumsum(nblocks)[:-1]))
    rank = np.arange(B) - class_row_start[labels_sorted]
    dst = class_slot_start[labels_sorted] + rank
    fpad[dst] = features[order]

    rows_core = J * P * S
    maps = []
    for k in range(N_CORES):
        fs = fpad[k * rows_core : (k + 1) * rows_core]
        # labW[p, j] = block_labels[(k*J + j)*128 + p]
        lw = np.ascontiguousarray(
            block_labels[k * J * P : (k + 1) * J * P].reshape(J, P).T
        )
        maps.append({"features": fs, "labels": lw, "centers": centers})
    return maps, J


def run(features, centers, labels, trace=False):
    """Run on 8 cores; returns (loss_scalar, BassKernelResults)."""
    maps, J = _prepare(features, centers, labels)
    nc = _build(J)
    res = run_bass_kernel_spmd(
        nc, maps, core_ids=list(range(N_CORES)), trace=trace
    )
    total = 0.0
    for r in res.results:
        total += float(np.asarray(r["out"]).astype(np.float64).sum())
    return np.float32(total / B), res


def kernel(features, centers, labels):
    loss, _ = run(features, centers, labels)
    return loss


# revision 31
# speedup vs baseline: 1.0016x; 1.0016x over previous
"""AdaptiveCenterLoss on 8 TRN2 NeuronCores.

loss = mean_i ||features[i] - centers[labels[i]]||^2
     with B=131072, D=256, C=1000.

Strategy (data-parallel, memory-bound):
  - host-side, sort rows by label and pack them into 8-row blocks, each
    block sharing one label; partial blocks are padded with rows equal to
    that class's center (contributing exactly 0 to the sum)
  - shard the padded blocks across 8 cores x 128 partitions as J 8-row
    "block-slots" per partition; per block-slot, ONE [128,1]-index
    indirect DMA gathers the 128 needed center rows (per-descriptor DGE
    cost makes per-row gathers ~16x more expensive on this HW)
  - block-slots are processed in pairs (one feature DMA + one DVE
    subtract with the two centers broadcast via a stride-0 4D AP + one
    ACT square+row-sum accumulate per pair); a trailing odd slot runs
    as a half tile, which also drains the pipeline faster
  - each core outputs per-tile partial sums; host sums and divides by B
"""

import numpy as np

import concourse.bacc as bacc
import concourse.bass as bass
import concourse.mybir as mybir
import concourse.tile as tile
from concourse.bass_utils import run_bass_kernel_spmd

B, D, C = 131072, 256, 1000
N_CORES = 8
P = 128   # SBUF partitions
S = 16    # rows per block (one label per block)

_nc_cache = {}


def _build(J):
    """Per-core graph for J block-slots per partition (J*8 rows each)."""
    if J in _nc_cache:
        return _nc_cache[J]
    splits = [1] * J
    acc_cols = sum(splits)

    nc = bacc.Bacc()
    feats = nc.declare_dram_parameter(
        "features", [J * P * S, D], mybir.dt.float32, isOutput=False
    )
    labels = nc.declare_dram_parameter("labels", [P, J], mybir.dt.int32, isOutput=False)
    centers = nc.declare_dram_parameter(
        "centers", [C, D], mybir.dt.float32, isOutput=False
    )
    out = nc.declare_dram_parameter(
        "out", [P, acc_cols], mybir.dt.float32, isOutput=True
    )

    # block-slot j, partition p, slot s <- feature row (j*128 + p)*8 + s
    fview = feats[:].rearrange("(j p s) d -> p j s d", p=P, s=S)

    with tile.TileContext(nc) as tc:
        with (
            tc.tile_pool(name="lab", bufs=1) as lab_pool,
            tc.tile_pool(name="f", bufs=4) as f_pool,
            tc.tile_pool(name="c", bufs=4) as c_pool,
            tc.tile_pool(name="acc", bufs=1) as acc_pool,
        ):
            lab = lab_pool.tile([P, J], mybir.dt.int32)
            nc.sync.dma_start(out=lab[:], in_=labels[:])
            acc = acc_pool.tile([P, acc_cols], mybir.dt.float32)
            col = 0
            for t in range(J):
                H = splits[t]
                SH = S // H
                f_t = f_pool.tile([P, S * D], mybir.dt.float32, tag="f")
                for h in range(H):
                    nc.sync.dma_start(
                        out=f_t[:, h * SH * D : (h + 1) * SH * D].rearrange(
                            "p (s d) -> p s d", s=SH
                        ),
                        in_=fview[:, t, h * SH : (h + 1) * SH, :],
                    )
                c_s = c_pool.tile([P, D], mybir.dt.float32, tag="c")
                nc.gpsimd.indirect_dma_start(
                    out=c_s[:],
                    out_offset=None,
                    in_=centers[:],
                    in_offset=bass.IndirectOffsetOnAxis(ap=lab[:, t : t + 1], axis=0),
                )
                c_b = (
                    c_s[:]
                    .rearrange("p (s d) -> p s d", s=1)
                    .to_broadcast([P, SH, D])
                )
                for h in range(H):
                    fh = f_t[:, h * SH * D : (h + 1) * SH * D]
                    nc.vector.tensor_tensor(
                        out=fh.rearrange("p (s d) -> p s d", s=SH),
                        in0=fh.rearrange("p (s d) -> p s d", s=SH),
                        in1=c_b,
                        op=mybir.AluOpType.subtract,
                    )
                    nc.scalar.activation(
                        out=fh,
                        in_=fh,
                        func=mybir.ActivationFunctionType.Square,
                        accum_out=acc[:, col : col + 1],
                    )
                    col += 1
            nc.sync.dma_start(out=out[:], in_=acc[:])
    nc.finalize()
    _nc_cache[J] = nc
    return nc


def _prepare(features, centers, labels):
    """Sort rows by label into padded S-row blocks; returns per-core maps + J."""
    features = np.ascontiguousarray(np.asarray(features), dtype=np.float32)
    centers = np.ascontiguousarray(np.asarray(centers), dtype=np.float32)
    labels = np.asarray(labels).astype(np.int32)

    counts = np.bincount(labels, minlength=C)          # [C]
    nblocks = -(-counts // S)                          # ceil(n_c / S) per class
    nb = int(nblocks.sum())
    group = N_CORES * P                                # blocks per slot across chip
    nb_pad = -(-nb // group) * group
    J = nb_pad // group                                # block-slots per partition

    # block labels, in sorted-class order; pad blocks use class 0
    block_labels = np.zeros(nb_pad, dtype=np.int32)
    block_labels[:nb] = np.repeat(np.arange(C, dtype=np.int32), nblocks)

    # every padded slot starts as its block's center row -> contributes 0
    fpad = centers[block_labels].repeat(S, axis=0).reshape(nb_pad * S, D)

    # scatter the real rows into their slots
    order = np.argsort(labels)
    labels_sorted = labels[order]
    class_row_start = np.concatenate(([0], np.cumsum(counts)[:-1]))
    class_slot_start = S * np.concatenate(([0], np.cumsum(nblocks)[:-1]))
    rank = np.arange(B) - class_row_start[labels_sorted]
    dst = class_slot_start[labels_sorted] + rank
    fpad[dst] = features[order]

    rows_core = J * P * S
    maps = []
    for k in range(N_CORES):
        fs = fpad[k * rows_core : (k + 1) * rows_core]
        # labW[p, j] = block_labels[(k*J + j)*128 + p]
        lw = np.ascontiguousarray(
            block_labels[k * J * P : (k + 1) * J * P].reshape(J, P).T
        )
        maps.append({"features": fs, "labels": lw, "centers": centers})
    return maps, J


def run(features, centers, labels, trace=False):
    """Run on 8 cores; returns (loss_scalar, BassKernelResults)."""
    maps, J = _prepare(features, centers, labels)
    nc = _build(J)
    res = run_bass_kernel_spmd(
        nc, maps, core_ids=list(range(N_CORES)), trace=trace
    )
    total = 0.0
    for r in res.results:
        total += float(np.asarray(r["out"]).astype(np.float64).sum())
    return np.float32(total / B), res


def kernel(features, centers, labels):
    loss, _ = run(features, centers, labels)
    return loss
